# revision 3
# baseline (speedup 1.0000x reference)
# Trainium2 Bass kernel for the CustomESN problem.
#
# Math (reference):
#   u_t = x_t @ W_in                                  [B, R]
#   s_{t+1} = 0.5*s_t + 0.5*tanh(s_t @ W_res + u_t)   (T steps, s_0 = 0)
#   out = s_T @ W_out                                 [B, O]
#
# Kernel substitution sigma_t = 2*s_t removes the output-side 0.5:
#   sigma_{t+1} = 0.5*sigma_t + tanh(sigma_t @ (0.5*W_res) + u_t)
#   out = sigma_T @ (0.5*W_out)
# so the per-step elementwise update is one fused DVE op:
#   sigma' = (sigma * 0.5) + tanh_result
#
# Sharding: data-parallel, batch 512 -> 8 cores x 64 rows, weights replicated.
#
# Per-core layout (b = 64 batch rows, R = 1024 reservoir, 8 chunks of 128):
#   sigmaT (state, transposed): SBUF [128, 8*64]; chunk ch col-block holds
#     sigma[b, ch*128 + p] at [p, ch*64 + b]. Used as matmul stationary lhsT.
#   W half-scaled, chunk-major: SBUF [128, 8*1024], Wl[p, ch*1024+n] = 0.5*W_res[ch*128+p, n]
#   preact PSUM [64, 512] per n-half: out = sum_ch sigmaT_ch.T @ Wl_ch + x_t @ W_in
#   tanh on ScalarE -> T_bn [64, 1024] bf16; transposed back to [128, 64] chunks
#   on TensorE (identity-matmul transpose); fused leak+add on VectorE.

import numpy as np
import ml_dtypes

BF16 = ml_dtypes.bfloat16

B = 512
T = 512
I = 64
R = 1024
O = 64
NCORES = 8
PB = B // NCORES  # 64 per-core batch rows
CH = R // 128     # 8 reservoir chunks

_prog_cache = {}


def _build_program(n_steps: int):
    import concourse.bacc as bacc
    import concourse.mybir as mybir
    import concourse.tile as tile
    from concourse.masks import make_identity

    f32 = mybir.dt.float32
    bf16 = mybir.dt.bfloat16
    AT = mybir.ActivationFunctionType
    ALU = mybir.AluOpType

    nc = bacc.Bacc("TRN2", target_bir_lowering=False, debug=False)

    xt_d = nc.dram_tensor("xt", [I, n_steps * PB], bf16, kind="ExternalInput")
    wl_d = nc.dram_tensor("wl", [128, CH * R], bf16, kind="ExternalInput")
    win_d = nc.dram_tensor("win", [I, R], bf16, kind="ExternalInput")
    wout_d = nc.dram_tensor("wout", [128, CH * O], bf16, kind="ExternalInput")
    y_d = nc.dram_tensor("y", [PB, O], f32, kind="ExternalOutput")

    with tile.TileContext(nc) as tc:
        with (
            tc.tile_pool(name="wpool", bufs=1) as wpool,
            tc.tile_pool(name="spool", bufs=1) as spool,
            tc.tile_pool(name="tpool", bufs=3) as tpool,
            tc.tile_pool(name="pa", bufs=4, space="PSUM") as pa_pool,
            tc.tile_pool(name="tp", bufs=2, space="PSUM") as tp_pool,
            tc.tile_pool(name="yp", bufs=1, space="PSUM") as yp_pool,
        ):
            xt_s = wpool.tile([I, n_steps * PB], bf16, tag="xt")
            wl_s = wpool.tile([128, CH * R], bf16, tag="wl")
            win_s = wpool.tile([I, R], bf16, tag="win")
            wout_s = wpool.tile([128, CH * O], bf16, tag="wout")
            ident = wpool.tile([64, 64], bf16, tag="ident")
            y_s = wpool.tile([PB, O], f32, tag="ys")

            nc.sync.dma_start(xt_s[:], xt_d[:])
            nc.sync.dma_start(wl_s[:], wl_d[:])
            nc.sync.dma_start(win_s[:], win_d[:])
            nc.sync.dma_start(wout_s[:], wout_d[:])
            make_identity(nc, ident[:])

            # ping-pong state, sigF = fp32 master, sigB = bf16 copy for PE
            sigF = [
                spool.tile([128, CH * PB], f32, tag=f"sigF{k}", name=f"sigF{k}")
                for k in range(2)
            ]
            sigB = [
                spool.tile([128, CH * PB], bf16, tag=f"sigB{k}", name=f"sigB{k}")
                for k in range(2)
            ]
            nc.vector.memset(sigF[0][:], 0.0)
            nc.vector.memset(sigB[0][:], 0.0)

            for t in range(n_steps):
                cur = t % 2
                nxt = (t + 1) % 2
                t_bn = tpool.tile([PB, R], bf16, tag="tbn")
                tp = tp_pool.tile([128, CH * PB], bf16, tag="tp")
                for h in (0, 1):
                    pa = pa_pool.tile([PB, 512], f32, tag="pa")
                    # input projection first: x always ready, keeps PE fed
                    nc.tensor.matmul(
                        pa[:],
                        xt_s[:, t * PB : (t + 1) * PB],
                        win_s[:, h * 512 : (h + 1) * 512],
                        start=True,
                        stop=False,
                    )
                    for ch in range(CH):
                        nc.tensor.matmul(
                            pa[:],
                            sigB[cur][:, ch * PB : (ch + 1) * PB],
                            wl_s[:, ch * R + h * 512 : ch * R + h * 512 + 512],
                            start=False,
                            stop=(ch == CH - 1),
                        )
                    nc.scalar.activation(t_bn[:, h * 512 : (h + 1) * 512], pa[:], AT.Tanh)
                    # transpose this half's 4 chunks back to [r, b] layout
                    for ch in range(4 * h, 4 * h + 4):
                        nc.tensor.transpose(
                            tp[:, ch * PB : (ch + 1) * PB],
                            t_bn[:, ch * 128 : (ch + 1) * 128],
                            ident[:],
                        )
                    # fused leak + add for this half's chunks, then bf16 copy
                    sl = slice(h * 4 * PB, (h + 1) * 4 * PB)
                    nc.vector.scalar_tensor_tensor(
                        out=sigF[nxt][:, sl],
                        in0=sigF[cur][:, sl],
                        scalar=0.5,
                        in1=tp[:, sl],
                        op0=ALU.mult,
                        op1=ALU.add,
                    )
                    nc.vector.tensor_copy(sigB[nxt][:, sl], sigF[nxt][:, sl])

            # output projection: y = sigma_T @ (0.5*W_out)
            fin = n_steps % 2
            yp = yp_pool.tile([PB, O], f32, tag="yp")
            for ch in range(CH):
                nc.tensor.matmul(
                    yp[:],
                    sigB[fin][:, ch * PB : (ch + 1) * PB],
                    wout_s[:, ch * O : (ch + 1) * O],
                    start=(ch == 0),
                    stop=(ch == CH - 1),
                )
            nc.scalar.copy(y_s[:], yp[:])
            nc.sync.dma_start(y_d[:], y_s[:])

    nc.compile()
    return nc


def _prep_inputs(input, W_reservoir, W_in, W_out, n_steps):
    wl = (0.5 * W_reservoir).reshape(CH, 128, R).transpose(1, 0, 2).reshape(128, CH * R)
    wl = np.ascontiguousarray(wl, dtype=np.float32).astype(BF16)
    win = np.ascontiguousarray(W_in, dtype=np.float32).astype(BF16)
    wout = (0.5 * W_out).reshape(CH, 128, O).transpose(1, 0, 2).reshape(128, CH * O)
    wout = np.ascontiguousarray(wout, dtype=np.float32).astype(BF16)

    in_maps = []
    for c in range(NCORES):
        xs = input[c * PB : (c + 1) * PB, :n_steps, :]  # [PB, n_steps, I]
        xt = np.ascontiguousarray(xs.transpose(2, 1, 0)).reshape(I, n_steps * PB)
        xt = xt.astype(BF16)
        in_maps.append({"xt": xt, "wl": wl, "win": win, "wout": wout})
    return in_maps


def kernel(input, W_reservoir, W_in, W_out, n_steps=T, trace=False):
    from concourse.bass_utils import run_bass_kernel_spmd

    input = np.asarray(input, dtype=np.float32)
    W_reservoir = np.asarray(W_reservoir, dtype=np.float32)
    W_in = np.asarray(W_in, dtype=np.float32)
    W_out = np.asarray(W_out, dtype=np.float32)

    if n_steps not in _prog_cache:
        _prog_cache[n_steps] = _build_program(n_steps)
    nc = _prog_cache[n_steps]

    in_maps = _prep_inputs(input, W_reservoir, W_in, W_out, n_steps)
    res = run_bass_kernel_spmd(
        nc, in_maps, core_ids=list(range(NCORES)), trace=trace
    )
    out = np.empty((B, O), dtype=np.float32)
    for c in range(NCORES):
        out[c * PB : (c + 1) * PB] = res.results[c]["y"]
    if trace:
        kernel._last_results = res
    return out


# revision 7
# speedup vs baseline: 1.3367x; 1.3367x over previous
# Trainium2 Bass kernel for the CustomESN problem.
#
# Math (reference):
#   u_t = x_t @ W_in                                  [B, R]
#   s_{t+1} = 0.5*s_t + 0.5*tanh(s_t @ W_res + u_t)   (T steps, s_0 = 0)
#   out = s_T @ W_out                                 [B, O]
#
# Kernel substitution sigma_t = 2*s_t removes the output-side 0.5:
#   sigma_{t+1} = 0.5*sigma_t + tanh(sigma_t @ (0.5*W_res) + u_t)
#   out = sigma_T @ (0.5*W_out)
# so the per-step elementwise update is one fused DVE op:
#   sigma' = (sigma * 0.5) + tanh_result
#
# Sharding: data-parallel, batch 512 -> 8 cores x 64 rows, weights replicated.
#
# Per-core layout (b = 64 batch rows, R = 1024 reservoir, 8 chunks of 128):
#   sigmaT (state, transposed): SBUF [128, 8*64]; chunk ch col-block holds
#     sigma[b, ch*128 + p] at [p, ch*64 + b]. Used as matmul stationary lhsT.
#   W half-scaled, chunk-major: SBUF [128, 8*1024], Wl[p, ch*1024+n] = 0.5*W_res[ch*128+p, n]
#   preact PSUM [64, 512] per n-half: out = sum_ch sigmaT_ch.T @ Wl_ch + x_t @ W_in
#   tanh on ScalarE -> T_bn [64, 1024] bf16; transposed back to [128, 64] chunks
#   on TensorE (identity-matmul transpose); fused leak+add on VectorE.

import numpy as np
import ml_dtypes

BF16 = ml_dtypes.bfloat16

B = 512
T = 512
I = 64
R = 1024
O = 64
NCORES = 8
PB = B // NCORES  # 64 per-core batch rows
CH = R // 128     # 8 reservoir chunks

_prog_cache = {}


def _build_program(n_steps: int, variant: int = 2):
    if variant == 2:
        return _build_program_v2(n_steps)
    return _build_program_v1(n_steps)


def _build_program_v2(n_steps: int):
    """Column-paired variant: for each reservoir chunk, the two n-halves run
    as two concurrent 64-col stationary groups (tile_position (0,0)/(0,64)),
    so the 128x128 PE array is fully used. preact PSUM is [128, 512]:
    rows 0:64 = n 0:511, rows 64:128 = n 512:1023 (same batch rows)."""
    import concourse.bacc as bacc
    import concourse.mybir as mybir
    import concourse.tile as tile

    f32 = mybir.dt.float32
    bf16 = mybir.dt.bfloat16
    AT = mybir.ActivationFunctionType
    ALU = mybir.AluOpType

    nc = bacc.Bacc("TRN2", target_bir_lowering=False, debug=False)

    xt_d = nc.dram_tensor("xt", [I, n_steps * PB], bf16, kind="ExternalInput")
    wl_d = nc.dram_tensor("wl", [128, CH * R], bf16, kind="ExternalInput")
    win_d = nc.dram_tensor("win", [I, R], bf16, kind="ExternalInput")
    wout_d = nc.dram_tensor("wout", [128, CH * O], bf16, kind="ExternalInput")
    id_d = nc.dram_tensor("ident", [128, 64], bf16, kind="ExternalInput")
    y_d = nc.dram_tensor("y", [PB, O], f32, kind="ExternalOutput")

    with tile.TileContext(nc) as tc:
        with (
            tc.tile_pool(name="wpool", bufs=1) as wpool,
            tc.tile_pool(name="spool", bufs=1) as spool,
            tc.tile_pool(name="tpool", bufs=3) as tpool,
            tc.tile_pool(name="pa", bufs=3, space="PSUM") as pa_pool,
            tc.tile_pool(name="tp", bufs=2, space="PSUM") as tp_pool,
            tc.tile_pool(name="yp", bufs=1, space="PSUM") as yp_pool,
        ):
            xt_s = wpool.tile([I, n_steps * PB], bf16, tag="xt")
            wl_s = wpool.tile([128, CH * R], bf16, tag="wl")
            win_s = wpool.tile([I, R], bf16, tag="win")
            wout_s = wpool.tile([128, CH * O], bf16, tag="wout")
            ident = wpool.tile([128, 64], bf16, tag="ident")
            y_s = wpool.tile([PB, O], f32, tag="ys")

            nc.sync.dma_start(xt_s[:], xt_d[:])
            nc.sync.dma_start(wl_s[:], wl_d[:])
            nc.sync.dma_start(win_s[:], win_d[:])
            nc.sync.dma_start(wout_s[:], wout_d[:])
            nc.sync.dma_start(ident[:], id_d[:])

            sigF = [
                spool.tile([128, CH * PB], f32, tag=f"sigF{k}", name=f"sigF{k}")
                for k in range(2)
            ]
            sigB = [
                spool.tile([128, CH * PB], bf16, tag=f"sigB{k}", name=f"sigB{k}")
                for k in range(2)
            ]
            nc.vector.memset(sigF[0][:], 0.0)
            nc.vector.memset(sigB[0][:], 0.0)

            for t in range(n_steps):
                cur = t % 2
                nxt = (t + 1) % 2
                t_bn = tpool.tile([128, 512], bf16, tag="tbn")
                tp = tp_pool.tile([128, CH * PB], bf16, tag="tp")
                pa = pa_pool.tile([128, 512], f32, tag="pa")
                # input projection pair (K=64)
                xsl = xt_s[:, t * PB : (t + 1) * PB]
                nc.tensor.matmul(
                    pa[0:64, :], xsl, win_s[:, 0:512],
                    start=True, stop=False, tile_position=(0, 0),
                )
                nc.tensor.matmul(
                    pa[64:128, :], xsl, win_s[:, 512:1024],
                    start=True, stop=False, tile_position=(0, 64),
                )
                # recurrence pairs (K=128)
                for ch in range(CH):
                    ssl = sigB[cur][:, ch * PB : (ch + 1) * PB]
                    last = ch == CH - 1
                    nc.tensor.matmul(
                        pa[0:64, :], ssl,
                        wl_s[:, ch * R : ch * R + 512],
                        start=False, stop=last, tile_position=(0, 0),
                    )
                    nc.tensor.matmul(
                        pa[64:128, :], ssl,
                        wl_s[:, ch * R + 512 : ch * R + 1024],
                        start=False, stop=last, tile_position=(0, 64),
                    )
                # tanh per half (lane-aligned in/out)
                nc.scalar.activation(t_bn[0:64, :], pa[0:64, :], AT.Tanh)
                nc.scalar.activation(t_bn[64:128, :], pa[64:128, :], AT.Tanh)
                # transpose chunks back to [r, b]; chunks 4-7 live on rows 64:128
                for ch in range(CH):
                    base = 0 if ch < 4 else 64
                    col = (ch % 4) * 128
                    nc.tensor.transpose(
                        tp[:, ch * PB : (ch + 1) * PB],
                        t_bn[base : base + 64, col : col + 128],
                        ident[base : base + 64, :],
                    )
                # fused leak+add and bf16 copy per half
                for h in (0, 1):
                    sl = slice(h * 4 * PB, (h + 1) * 4 * PB)
                    nc.vector.scalar_tensor_tensor(
                        out=sigF[nxt][:, sl],
                        in0=sigF[cur][:, sl],
                        scalar=0.5,
                        in1=tp[:, sl],
                        op0=ALU.mult,
                        op1=ALU.add,
                    )
                    nc.vector.tensor_copy(sigB[nxt][:, sl], sigF[nxt][:, sl])

            fin = n_steps % 2
            yp = yp_pool.tile([PB, O], f32, tag="yp")
            for ch in range(CH):
                nc.tensor.matmul(
                    yp[:],
                    sigB[fin][:, ch * PB : (ch + 1) * PB],
                    wout_s[:, ch * O : (ch + 1) * O],
                    start=(ch == 0),
                    stop=(ch == CH - 1),
                )
            nc.scalar.copy(y_s[:], yp[:])
            nc.sync.dma_start(y_d[:], y_s[:])

    nc.compile()
    return nc


def _build_program_v1(n_steps: int):
    import concourse.bacc as bacc
    import concourse.mybir as mybir
    import concourse.tile as tile
    from concourse.masks import make_identity

    f32 = mybir.dt.float32
    bf16 = mybir.dt.bfloat16
    AT = mybir.ActivationFunctionType
    ALU = mybir.AluOpType

    nc = bacc.Bacc("TRN2", target_bir_lowering=False, debug=False)

    xt_d = nc.dram_tensor("xt", [I, n_steps * PB], bf16, kind="ExternalInput")
    wl_d = nc.dram_tensor("wl", [128, CH * R], bf16, kind="ExternalInput")
    win_d = nc.dram_tensor("win", [I, R], bf16, kind="ExternalInput")
    wout_d = nc.dram_tensor("wout", [128, CH * O], bf16, kind="ExternalInput")
    y_d = nc.dram_tensor("y", [PB, O], f32, kind="ExternalOutput")

    with tile.TileContext(nc) as tc:
        with (
            tc.tile_pool(name="wpool", bufs=1) as wpool,
            tc.tile_pool(name="spool", bufs=1) as spool,
            tc.tile_pool(name="tpool", bufs=3) as tpool,
            tc.tile_pool(name="pa", bufs=4, space="PSUM") as pa_pool,
            tc.tile_pool(name="tp", bufs=2, space="PSUM") as tp_pool,
            tc.tile_pool(name="yp", bufs=1, space="PSUM") as yp_pool,
        ):
            xt_s = wpool.tile([I, n_steps * PB], bf16, tag="xt")
            wl_s = wpool.tile([128, CH * R], bf16, tag="wl")
            win_s = wpool.tile([I, R], bf16, tag="win")
            wout_s = wpool.tile([128, CH * O], bf16, tag="wout")
            ident = wpool.tile([64, 64], bf16, tag="ident")
            y_s = wpool.tile([PB, O], f32, tag="ys")

            nc.sync.dma_start(xt_s[:], xt_d[:])
            nc.sync.dma_start(wl_s[:], wl_d[:])
            nc.sync.dma_start(win_s[:], win_d[:])
            nc.sync.dma_start(wout_s[:], wout_d[:])
            make_identity(nc, ident[:])

            # ping-pong state, sigF = fp32 master, sigB = bf16 copy for PE
            sigF = [
                spool.tile([128, CH * PB], f32, tag=f"sigF{k}", name=f"sigF{k}")
                for k in range(2)
            ]
            sigB = [
                spool.tile([128, CH * PB], bf16, tag=f"sigB{k}", name=f"sigB{k}")
                for k in range(2)
            ]
            nc.vector.memset(sigF[0][:], 0.0)
            nc.vector.memset(sigB[0][:], 0.0)

            for t in range(n_steps):
                cur = t % 2
                nxt = (t + 1) % 2
                t_bn = tpool.tile([PB, R], bf16, tag="tbn")
                tp = tp_pool.tile([128, CH * PB], bf16, tag="tp")
                for h in (0, 1):
                    pa = pa_pool.tile([PB, 512], f32, tag="pa")
                    # input projection first: x always ready, keeps PE fed
                    nc.tensor.matmul(
                        pa[:],
                        xt_s[:, t * PB : (t + 1) * PB],
                        win_s[:, h * 512 : (h + 1) * 512],
                        start=True,
                        stop=False,
                    )
                    for ch in range(CH):
                        nc.tensor.matmul(
                            pa[:],
                            sigB[cur][:, ch * PB : (ch + 1) * PB],
                            wl_s[:, ch * R + h * 512 : ch * R + h * 512 + 512],
                            start=False,
                            stop=(ch == CH - 1),
                        )
                    nc.scalar.activation(t_bn[:, h * 512 : (h + 1) * 512], pa[:], AT.Tanh)
                    # transpose this half's 4 chunks back to [r, b] layout
                    for ch in range(4 * h, 4 * h + 4):
                        nc.tensor.transpose(
                            tp[:, ch * PB : (ch + 1) * PB],
                            t_bn[:, ch * 128 : (ch + 1) * 128],
                            ident[:],
                        )
                    # fused leak + add for this half's chunks, then bf16 copy
                    sl = slice(h * 4 * PB, (h + 1) * 4 * PB)
                    nc.vector.scalar_tensor_tensor(
                        out=sigF[nxt][:, sl],
                        in0=sigF[cur][:, sl],
                        scalar=0.5,
                        in1=tp[:, sl],
                        op0=ALU.mult,
                        op1=ALU.add,
                    )
                    nc.vector.tensor_copy(sigB[nxt][:, sl], sigF[nxt][:, sl])

            # output projection: y = sigma_T @ (0.5*W_out)
            fin = n_steps % 2
            yp = yp_pool.tile([PB, O], f32, tag="yp")
            for ch in range(CH):
                nc.tensor.matmul(
                    yp[:],
                    sigB[fin][:, ch * PB : (ch + 1) * PB],
                    wout_s[:, ch * O : (ch + 1) * O],
                    start=(ch == 0),
                    stop=(ch == CH - 1),
                )
            nc.scalar.copy(y_s[:], yp[:])
            nc.sync.dma_start(y_d[:], y_s[:])

    nc.compile()
    return nc


def _prep_inputs(input, W_reservoir, W_in, W_out, n_steps, variant=2):
    wl = (0.5 * W_reservoir).reshape(CH, 128, R).transpose(1, 0, 2).reshape(128, CH * R)
    wl = np.ascontiguousarray(wl, dtype=np.float32).astype(BF16)
    win = np.ascontiguousarray(W_in, dtype=np.float32).astype(BF16)
    wout = (0.5 * W_out).reshape(CH, 128, O).transpose(1, 0, 2).reshape(128, CH * O)
    wout = np.ascontiguousarray(wout, dtype=np.float32).astype(BF16)

    ident = np.vstack([np.eye(64), np.eye(64)]).astype(BF16)  # [128, 64]

    in_maps = []
    for c in range(NCORES):
        xs = input[c * PB : (c + 1) * PB, :n_steps, :]  # [PB, n_steps, I]
        xt = np.ascontiguousarray(xs.transpose(2, 1, 0)).reshape(I, n_steps * PB)
        xt = xt.astype(BF16)
        m = {"xt": xt, "wl": wl, "win": win, "wout": wout}
        if variant == 2:
            m["ident"] = ident
        in_maps.append(m)
    return in_maps


def kernel(input, W_reservoir, W_in, W_out, n_steps=T, trace=False, variant=2):
    from concourse.bass_utils import run_bass_kernel_spmd

    input = np.asarray(input, dtype=np.float32)
    W_reservoir = np.asarray(W_reservoir, dtype=np.float32)
    W_in = np.asarray(W_in, dtype=np.float32)
    W_out = np.asarray(W_out, dtype=np.float32)

    key = (n_steps, variant)
    if key not in _prog_cache:
        _prog_cache[key] = _build_program(n_steps, variant)
    nc = _prog_cache[key]

    in_maps = _prep_inputs(input, W_reservoir, W_in, W_out, n_steps, variant)
    res = run_bass_kernel_spmd(
        nc, in_maps, core_ids=list(range(NCORES)), trace=trace
    )
    out = np.empty((B, O), dtype=np.float32)
    for c in range(NCORES):
        out[c * PB : (c + 1) * PB] = res.results[c]["y"]
    if trace:
        kernel._last_results = res
    return out
